# revision 12
# baseline (speedup 1.0000x reference)
"""Trainium2 Bass kernel for nn_MultiHeadAttention_36112085025201.

Multi-head attention, B=2, S=4096, D=512, H=8 heads, Dh=64.
Sharding: 8 cores = 2 (batch) x 4 (head-pairs). Each core computes its
batch's attention for 2 heads plus that head-slice's contribution to the
output projection; the host sums the 4 partial projections per batch and
adds the (bv@wo + bo) bias row.

Per-core algorithm (scores bf16, PV + out-proj fp8 DoubleRow, fp32 PSUM):
  - qT,kT [128,S] = w.T @ x.T; bias added during the PSUM eviction on the
    ACT engine (idle during projections).
  - v [S,128] evicted to fp8 in DoubleRow pair layout vext_dr[h]
    [128, NPAIR, 2, VWP] with a trailing ones column (denominator trick).
  - attention runs as one flattened loop over (query block jb, key pair p)
    slots; each slot: 4 score MMs [K=64,M=128,N=512] ordered h0,h1 per
    chunk so the row-disjoint head MMs co-execute; exp of each [128,1024]
    scores tile alternates ACT (true exp -> fp8) / DVE (Schraudolph
    affine -> int8 = fp8e4 bits); PV is one fp8 DoubleRow MM per head
    covering the pair (K_eff=256). PV emission is deferred 2 slots and
    the out-projection/normalize work of finished blocks is slotted into
    the next block's pairs, so the PE never idles at block boundaries
    (idle >3.4us re-throttles the PE clock to 1.2GHz).
  - normalize: denominator row to partition 0, DVE approx reciprocal,
    GPSIMD partition broadcast, DVE multiply -> onormT bf16.
  - out projection per 128-row tile, deferred behind the next block's
    pairs; eviction copies alternate DVE/ACT; output DMAs split across
    queues (per-queue DMA is descriptor-rate-bound at ~2KB/80ns).
"""

import numpy as np
from collections import deque
from contextlib import ExitStack

import ml_dtypes
import concourse.tile as tile
from concourse import bacc, mybir
from concourse.bass_utils import run_bass_kernel_spmd

# Problem constants (hardcoded per harness contract).
B, D = 2, 512
S = 4096               # sequence length (overridable for sim harnesses)
H, Dh = 8, 64
SCALE = Dh ** -0.5
N_CORES = 8
HL = 2                 # heads per core
CW = HL * Dh           # 128 local head columns per core
NK = D // 128          # 4 contraction chunks for projections
VW = Dh + 1            # v width incl. ones column
VWP = 80               # padded v width (DoubleRow weight step must be %16)

BF16 = mybir.dt.bfloat16
F32 = mybir.dt.float32
FP8 = mybir.dt.float8e4
I8 = mybir.dt.int8
EXP = mybir.ActivationFunctionType.Exp
IDENT = mybir.ActivationFunctionType.Identity
ALU = mybir.AluOpType
DR = mybir.MatmulPerfMode.DoubleRow

# Schraudolph exp in fp8e4(m3) bit domain: i8 = trunc(x*A8 + B8);
# bitcast to fp8. A8 folds the attention scale; B8 centers the
# log2(1+m) spline error and the truncation bias (constant-factor
# errors cancel exactly in softmax).
A8 = float(8 * np.log2(np.e) * SCALE)
B8 = float(7 * 8 - 8 * 0.0430 + 0.45)

# exp engine assignment by flat tile index: ACT (real exp ~1.11us/tile)
# vs DVE (Schraudolph ~1.22us/tile). GPSIMD cannot read PSUM.
PATTERN = "ADADADAADADA"  # 7xA, 5xD per 12


def _build_body(ctx: ExitStack, tc: "tile.TileContext", io: dict, dbg: dict | None = None):
    nc = tc.nc
    xT, wq, wk, wv, wo = io["xT"], io["wq"], io["wk"], io["wv"], io["wo"]
    bq, bk, out = io["bq"], io["bk"], io["out"]

    NSQ = S // 512     # query blocks
    NST = S // 128     # key chunks
    NPAIR = NST // 2   # key chunk pairs (DoubleRow granularity)

    const = ctx.enter_context(tc.tile_pool(name="const", bufs=1))
    persist = ctx.enter_context(tc.tile_pool(name="persist", bufs=1))

    # Persistent SBUF arrays.
    xT_sb = [persist.tile([128, S], BF16, tag=f"xT{k}", name=f"xT{k}") for k in range(NK)]
    qT_sb = persist.tile([128, S], BF16, tag="qT")
    kT_sb = persist.tile([128, S], BF16, tag="kT")
    vext = [persist.tile([128, NPAIR, 2, VWP], FP8, tag=f"vext{h}", name=f"vext{h}")
            for h in range(HL)]
    onormT = persist.tile([128, S], BF16, tag="onormT")

    wq_all = const.tile([128, NK, CW], BF16, tag="wq")
    wk_all = const.tile([128, NK, CW], BF16, tag="wk")
    wv_all = const.tile([128, NK, CW], BF16, tag="wv")
    wq_sb = [wq_all[:, k, :] for k in range(NK)]
    wk_sb = [wk_all[:, k, :] for k in range(NK)]
    wv_sb = [wv_all[:, k, :] for k in range(NK)]
    wo_sb = const.tile([128, D], BF16, tag="wo")
    bq_sb = const.tile([CW, 1], F32, tag="bq")
    bk_sb = const.tile([CW, 1], F32, tag="bk")

    # Input DMAs, ordered so the kT projection can start earliest:
    # wk + first xT block gate the first matmul. Weights arrive
    # host-pre-shuffled as [128, NK*CW] so each is one DMA with 1KB
    # partition lines (the DMA queues are descriptor-rate-bound).
    nc.sync.dma_start(wk_all[:], wk[:, :].rearrange("p (a b) -> p a b", a=NK))
    CC = min(1024, S)
    for k in range(NK):
        for cc in range(0, CC, 512):
            nc.sync.dma_start(xT_sb[k][:, cc:cc + 512],
                              xT[128 * k:128 * (k + 1), cc:cc + 512])
    nc.sync.dma_start(wv_all[:], wv[:, :].rearrange("p (a b) -> p a b", a=NK))
    nc.sync.dma_start(wq_all[:], wq[:, :].rearrange("p (a b) -> p a b", a=NK))
    nc.sync.dma_start(wo_sb[:], wo[:, :])
    nc.sync.dma_start(bq_sb[:], bq[:, :])
    nc.sync.dma_start(bk_sb[:], bk[:, :])
    for jp in range(1, S // CC):
        for k in range(NK):
            for cc in range(CC * jp, CC * (jp + 1), 512):
                nc.sync.dma_start(xT_sb[k][:, cc:cc + 512],
                                  xT[128 * k:128 * (k + 1), cc:cc + 512])

    # PSUM (8 banks): pmm 3x[128,1024] = 6 banks, pacc 2x[65,512] = 2.
    pmm = ctx.enter_context(tc.tile_pool(name="pmm", bufs=3, space="PSUM"))
    pacc = ctx.enter_context(tc.tile_pool(name="pacc", bufs=1, space="PSUM"))

    expp = ctx.enter_context(tc.tile_pool(name="expp", bufs=4))
    rp = ctx.enter_context(tc.tile_pool(name="rp", bufs=4))
    outp = ctx.enter_context(tc.tile_pool(name="outp", bufs=3))

    # Projections: one 1024-col block = 8 MMs + one ACT bias-add eviction.
    def qk_proj_block(w_sb, b_sb, dst, jp):
        ps = pmm.tile([128, 1024], F32, tag="mm")
        for k in range(NK):
            for jj in range(2):
                nc.tensor.matmul(ps[:, 512 * jj:512 * (jj + 1)], w_sb[k][:],
                                 xT_sb[k][:, 1024 * jp + 512 * jj:1024 * jp + 512 * (jj + 1)],
                                 start=(k == 0), stop=(k == NK - 1))
        nc.scalar.activation(dst[:, 1024 * jp:1024 * (jp + 1)], ps[:],
                             IDENT, bias=b_sb[:])

    for jp in range(S // 1024):
        qk_proj_block(wk_sb, bk_sb, kT_sb, jp)

    # v projection in normal orientation [s, c], split per head into the
    # DoubleRow pair layout (fp8) with trailing ones columns. Pairs 0-3
    # are emitted up front; the rest interleave into the first attention
    # slots (PV for pair p only runs ~2 slots after slot p).
    def v_proj_pair(tp):
        ps = pmm.tile([128, 1024], F32, tag="mm", name="vps")
        for tt in range(2):
            t = 2 * tp + tt
            for k in range(NK):
                nc.tensor.matmul(ps[:, 512 * tt:512 * tt + CW],
                                 xT_sb[k][:, 128 * t:128 * (t + 1)], wv_sb[k][:],
                                 start=(k == 0), stop=(k == NK - 1))
        for tt in range(2):
            for h in range(HL):
                nc.vector.tensor_copy(vext[h][:, tp, tt, 0:Dh],
                                      ps[:, 512 * tt + Dh * h:512 * tt + Dh * (h + 1)])

    for h in range(HL):
        nc.vector.memset(vext[h][:], 1.0)
    NV_PRE = min(4, NPAIR)
    for tp in range(NV_PRE):
        v_proj_pair(tp)

    qk_proj_block(wq_sb, bq_sb, qT_sb, 0)  # later blocks prefetched in-loop

    # ---- Flattened attention loop ----
    def emit_pv(e_tiles, p, po):
        for h in range(HL):
            nc.tensor.matmul(
                po[h][:],
                vext[h][:, p, :, 0:VW],
                e_tiles[h][:].rearrange("p (a b) -> p a b", a=2),
                start=(p == 0), stop=(p == NPAIR - 1),
                perf_mode=DR)

    def emit_normalize(po, jb):
        q0 = 512 * jb
        r0, r, rb = {}, {}, {}
        for h in range(HL):
            # NB: reciprocal_approx_* mis-executes at base partition != 0;
            # move the denominator row (partition 64) to partition 0.
            r0[h] = rp.tile([1, 512], F32, tag="r0", name=f"r0{h}")
            nc.vector.tensor_copy(r0[h][:], po[h][Dh:VW, :])
        for h in range(HL):
            r[h] = rp.tile([1, 512], F32, tag="r", name=f"r{h}")
            nc.vector.reciprocal_approx_fast(r[h][:], r0[h][:])
        for h in range(HL):
            rb[h] = rp.tile([Dh, 512], F32, tag="rb", name=f"rb{h}")
            nc.gpsimd.partition_broadcast(rb[h][:], r[h][:])
        for h in range(HL):
            nc.vector.tensor_mul(onormT[Dh * h:Dh * (h + 1), q0:q0 + 512],
                                 po[h][0:Dh, :], rb[h][:])

    def emit_op(jb_, st):
        sq0 = 512 * jb_ + 128 * st
        pf = pmm.tile([128, 1024], F32, tag="mm", name="pf")
        nc.tensor.matmul(pf[:, 0:512], onormT[:, sq0:sq0 + 128], wo_sb[:],
                         start=True, stop=True)
        ob = outp.tile([128, 512], F32, tag="ob")
        if (jb_ * 4 + st) % 2 == 0:
            nc.vector.tensor_copy(ob[:], pf[:, 0:512])
        else:
            nc.scalar.copy(ob[:], pf[:, 0:512])
        nc.sync.dma_start(out[sq0:sq0 + 64, :], ob[0:64, :])
        nc.sync.dma_start(out[sq0 + 64:sq0 + 128, :], ob[64:128, :])

    pending_pv = deque()   # (e_tiles, pair, po, jb)
    pending_op = deque()   # (jb, st)
    op_ready = 10 ** 9
    tile_idx = 0
    g = 0

    def pop_pv():
        nonlocal op_ready
        e_t, pp, ppo, pjb = pending_pv.popleft()
        emit_pv(e_t, pp, ppo)
        if pp == NPAIR - 1:
            emit_normalize(ppo, pjb)
            pending_op.extend((pjb, st) for st in range(4))
            op_ready = g + 3

    for jb in range(NSQ):
        q0 = 512 * jb
        po = {h: pacc.tile([VW, 512], F32, tag=f"acc{h}", name=f"po{h}")
              for h in range(HL)}
        for p in range(NPAIR):
            s = {h: pmm.tile([128, 1024], F32, tag="mm", name=f"s{h}")
                 for h in range(HL)}
            # score MMs: heads alternate so their row-disjoint (64-row)
            # MMs co-execute in the PE array.
            for i in range(2):
                c = 2 * p + i
                for h in range(HL):
                    nc.tensor.matmul(s[h][:, 512 * i:512 * (i + 1)],
                                     kT_sb[Dh * h:Dh * (h + 1), 128 * c:128 * (c + 1)],
                                     qT_sb[Dh * h:Dh * (h + 1), q0:q0 + 512],
                                     start=True, stop=True)
            e_cur = {}
            for h in range(HL):
                e_cur[h] = expp.tile([128, 1024], FP8, tag=f"e{h}", bufs=4,
                                     name=f"e{h}")
                eng = PATTERN[tile_idx % len(PATTERN)]
                tile_idx += 1
                if eng == "A":
                    nc.scalar.activation(e_cur[h][:], s[h][:], EXP, scale=float(SCALE))
                else:
                    nc.vector.tensor_scalar(e_cur[h][:].bitcast(I8), s[h][:],
                                            A8, B8, ALU.mult, ALU.add)
            pending_pv.append((e_cur, p, po, jb))
            if len(pending_pv) > 2:
                pop_pv()
            if pending_op and g >= op_ready:
                for _ in range(2):
                    if pending_op:
                        emit_op(*pending_op.popleft())
            if jb == 0 and NV_PRE + p < NPAIR:
                v_proj_pair(NV_PRE + p)
            if p == NPAIR // 2 and jb % 2 == 0 and jb + 2 < NSQ:
                qk_proj_block(wq_sb, bq_sb, qT_sb, (jb + 2) // 2)
            g += 1

    while pending_pv:
        pop_pv()
    while pending_op:
        emit_op(*pending_op.popleft())

    if dbg:
        for name, sb in (("qT", qT_sb), ("kT", kT_sb)):
            if name in dbg:
                nc.sync.dma_start(dbg[name][:, :], sb[:])


def build_nc():
    nc = bacc.Bacc("TRN2", target_bir_lowering=False, debug=False,
                   enable_asserts=False, num_devices=N_CORES)
    NK_ = D // 128
    io = {
        "xT": nc.dram_tensor("xT", [D, S], BF16, kind="ExternalInput").ap(),
        "wq": nc.dram_tensor("wq", [128, NK_ * CW], BF16, kind="ExternalInput").ap(),
        "wk": nc.dram_tensor("wk", [128, NK_ * CW], BF16, kind="ExternalInput").ap(),
        "wv": nc.dram_tensor("wv", [128, NK_ * CW], BF16, kind="ExternalInput").ap(),
        "wo": nc.dram_tensor("wo", [CW, D], BF16, kind="ExternalInput").ap(),
        "bq": nc.dram_tensor("bq", [CW, 1], F32, kind="ExternalInput").ap(),
        "bk": nc.dram_tensor("bk", [CW, 1], F32, kind="ExternalInput").ap(),
        "out": nc.dram_tensor("out", [S, D], F32, kind="ExternalOutput").ap(),
    }
    with tile.TileContext(nc) as tc, ExitStack() as ctx:
        _build_body(ctx, tc, io)
    nc.compile()
    return nc


def _shuf_w(w):
    # [D, CW] -> [128, NK*CW]: row 128k+p lands at [p, k*CW:(k+1)*CW]
    nk = w.shape[0] // 128
    return np.ascontiguousarray(
        w.reshape(nk, 128, -1).transpose(1, 0, 2).reshape(128, -1))


def make_in_maps(x, wq, bq, wk, bk, wv, bv, wo, bo):
    """Shard the full inputs across the 8 cores (host-side marshalling)."""
    bf16 = ml_dtypes.bfloat16
    fp8 = ml_dtypes.float8_e4m3
    in_maps = []
    for c in range(N_CORES):
        b, hp = divmod(c, 4)
        cs = slice(CW * hp, CW * (hp + 1))
        xT = np.ascontiguousarray(x[b, :S].T).astype(bf16)

        in_maps.append({
            "xT": xT,
            "wq": _shuf_w(wq[:, cs]).astype(bf16),
            "wk": _shuf_w(wk[:, cs]).astype(bf16),
            "wv": _shuf_w(wv[:, cs]).astype(bf16),
            "wo": np.ascontiguousarray(wo[cs, :]).astype(bf16),
            "bq": np.ascontiguousarray(bq[cs].reshape(CW, 1)).astype(np.float32),
            "bk": np.ascontiguousarray(bk[cs].reshape(CW, 1)).astype(np.float32),
        })
    return in_maps


_CACHE = {}


def _get_nc():
    if "nc" not in _CACHE:
        _CACHE["nc"] = build_nc()
    return _CACHE["nc"]


def run_sharded(nc, in_maps, **kwargs):
    return run_bass_kernel_spmd(nc, in_maps, core_ids=list(range(N_CORES)), **kwargs)


def gather(results, bv, wo, bo):
    # v is projected without bias on-device; softmax rows sum to 1, so
    # out = device_out_sum + (bv @ wo + bo).
    bias_row = (bv.astype(np.float64) @ wo.astype(np.float64)
                + bo.astype(np.float64)).astype(np.float32)
    out = np.zeros((B, S, D), np.float32)
    for c in range(N_CORES):
        out[c // 4] += results[c]["out"]
    out += bias_row
    return out


def kernel(x, wq, bq, wk, bk, wv, bv, wo, bo):
    x, wq, bq, wk, bk, wv, bv, wo, bo = (
        np.asarray(a, np.float32) for a in (x, wq, bq, wk, bk, wv, bv, wo, bo))
    nc = _get_nc()
    in_maps = make_in_maps(x, wq, bq, wk, bk, wv, bv, wo, bo)
    res = run_sharded(nc, in_maps)
    return gather(res.results, bv, wo, bo)


# revision 13
# speedup vs baseline: 1.0834x; 1.0834x over previous
"""Trainium2 Bass kernel for nn_MultiHeadAttention_36112085025201.

Multi-head attention, B=2, S=4096, D=512, H=8 heads, Dh=64.
Sharding: 8 cores = 2 (batch) x 4 (head-pairs). Each core computes its
batch's attention for 2 heads plus that head-slice's contribution to the
output projection; the host sums the 4 partial projections per batch and
adds the (bv@wo + bo) bias row.

Per-core algorithm (scores bf16, PV in fp8 DoubleRow, fp32 PSUM):
  - qT,kT [128,S] = w.T @ x.T; bias added during the PSUM eviction on the
    ACT engine (idle during projections).
  - v [S,128] evicted to fp8 in DoubleRow pair layout vext[h]
    [128, NPAIR, 2, VWP] with a trailing ones column (denominator trick).
  - streaming attention per (512-wide query block jb): per key-chunk pair
    p, 4 score MMs [K=64,M=128,N=512] ordered h0,h1 per chunk so the two
    heads' row-disjoint MMs co-execute in the PE array; exp of each
    [128,1024] scores tile alternates ACT (true exp -> fp8) / DVE
    (Schraudolph affine -> int8 = fp8e4 bits); PV is one fp8 DoubleRow MM
    per head covering the pair (K_eff=256), accumulating [v | ones].T @ e.
    PV emission is deferred one pair so exp latency hides behind the next
    pair's score MMs.
  - normalize: denominator row to partition 0 (DVE), DVE approx
    reciprocal, GPSIMD partition broadcast, DVE multiply -> onormT bf16.
  - out projection per 128-row tile, deferred one block to keep the PE
    fed across block boundaries; eviction copies alternate DVE/ACT; the
    bias row is added on the host.
"""

import numpy as np
from contextlib import ExitStack

import ml_dtypes
import concourse.tile as tile
from concourse import bacc, mybir
from concourse.bass_utils import run_bass_kernel_spmd

# Problem constants (hardcoded per harness contract).
B, D = 2, 512
S = 4096               # sequence length (overridable for sim harnesses)
H, Dh = 8, 64
SCALE = Dh ** -0.5
N_CORES = 8
HL = 2                 # heads per core
CW = HL * Dh           # 128 local head columns per core
NK = D // 128          # 4 contraction chunks for projections
VW = Dh + 1            # v width incl. ones column
VWP = 80               # padded v width (DoubleRow weight step must be %16)

BF16 = mybir.dt.bfloat16
F32 = mybir.dt.float32
FP8 = mybir.dt.float8e4
I8 = mybir.dt.int8
EXP = mybir.ActivationFunctionType.Exp
IDENT = mybir.ActivationFunctionType.Identity
ALU = mybir.AluOpType
DR = mybir.MatmulPerfMode.DoubleRow

# Schraudolph exp in fp8e4(m3) bit domain: i8 = trunc(x*A8 + B8);
# bitcast to fp8. A8 folds the attention scale; B8 centers the
# log2(1+m) spline error and the truncation bias (constant-factor
# errors cancel exactly in softmax).
A8 = float(8 * np.log2(np.e) * SCALE)
B8 = float(7 * 8 - 8 * 0.0430 + 0.45)

# exp engine assignment by flat tile index: ACT (real exp ~1.11us/tile)
# vs DVE (Schraudolph ~1.22us/tile). GPSIMD cannot read PSUM.
PATTERN = "ADADADAADADA"  # 7xA, 5xD per 12


def _build_body(ctx: ExitStack, tc: "tile.TileContext", io: dict, dbg: dict | None = None):
    nc = tc.nc
    xT, wq, wk, wv, wo = io["xT"], io["wq"], io["wk"], io["wv"], io["wo"]
    bq, bk, out = io["bq"], io["bk"], io["out"]

    NSQ = S // 512     # query blocks
    NST = S // 128     # key chunks
    NPAIR = NST // 2   # key chunk pairs (DoubleRow granularity)

    const = ctx.enter_context(tc.tile_pool(name="const", bufs=1))
    persist = ctx.enter_context(tc.tile_pool(name="persist", bufs=1))

    # Persistent SBUF arrays.
    xT_sb = [persist.tile([128, S], BF16, tag=f"xT{k}", name=f"xT{k}") for k in range(NK)]
    qT_sb = persist.tile([128, S], BF16, tag="qT")
    kT_sb = persist.tile([128, S], BF16, tag="kT")
    vext = [persist.tile([128, NPAIR, 2, VWP], FP8, tag=f"vext{h}", name=f"vext{h}")
            for h in range(HL)]
    onormT = persist.tile([128, S], BF16, tag="onormT")

    wq_all = const.tile([128, NK, CW], BF16, tag="wq")
    wk_all = const.tile([128, NK, CW], BF16, tag="wk")
    wv_all = const.tile([128, NK, CW], BF16, tag="wv")
    wq_sb = [wq_all[:, k, :] for k in range(NK)]
    wk_sb = [wk_all[:, k, :] for k in range(NK)]
    wv_sb = [wv_all[:, k, :] for k in range(NK)]
    wo_sb = const.tile([128, D], BF16, tag="wo")
    bq_sb = const.tile([CW, 1], F32, tag="bq")
    bk_sb = const.tile([CW, 1], F32, tag="bk")

    # Input DMAs, ordered so the kT projection can start earliest.
    # Weights arrive host-pre-shuffled as [128, NK*CW] (1KB partition
    # lines, one DMA each); the wk DMA is partition-split across queues
    # (per-queue DMA is byte-rate-bound at ~24GB/s).
    for pp in range(0, 128, 32):
        nc.sync.dma_start(wk_all[pp:pp + 32, :],
                          wk[pp:pp + 32, :].rearrange("p (a b) -> p a b", a=NK))
    CC = min(1024, S)
    for k in range(NK):
        for cc in range(0, CC, 512):
            nc.sync.dma_start(xT_sb[k][:, cc:cc + 512],
                              xT[128 * k:128 * (k + 1), cc:cc + 512])
    nc.sync.dma_start(wv_all[:], wv[:, :].rearrange("p (a b) -> p a b", a=NK))
    nc.sync.dma_start(wq_all[:], wq[:, :].rearrange("p (a b) -> p a b", a=NK))
    nc.sync.dma_start(wo_sb[:], wo[:, :])
    nc.sync.dma_start(bq_sb[:], bq[:, :])
    nc.sync.dma_start(bk_sb[:], bk[:, :])
    for jp in range(1, S // CC):
        for k in range(NK):
            for cc in range(CC * jp, CC * (jp + 1), 512):
                nc.sync.dma_start(xT_sb[k][:, cc:cc + 512],
                                  xT[128 * k:128 * (k + 1), cc:cc + 512])

    # PSUM (8 banks): pmm 3x[128,1024] = 6 banks, pacc 2x[65,512] = 2.
    pmm = ctx.enter_context(tc.tile_pool(name="pmm", bufs=3, space="PSUM"))
    pacc = ctx.enter_context(tc.tile_pool(name="pacc", bufs=1, space="PSUM"))

    expp = ctx.enter_context(tc.tile_pool(name="expp", bufs=3))
    rp = ctx.enter_context(tc.tile_pool(name="rp", bufs=4))
    outp = ctx.enter_context(tc.tile_pool(name="outp", bufs=3))

    # Projections: one 1024-col block = 8 MMs + one ACT bias-add eviction.
    def qk_proj_block(w_sb, b_sb, dst, jp):
        ps = pmm.tile([128, 1024], F32, tag="mm")
        for k in range(NK):
            for jj in range(2):
                nc.tensor.matmul(ps[:, 512 * jj:512 * (jj + 1)], w_sb[k][:],
                                 xT_sb[k][:, 1024 * jp + 512 * jj:1024 * jp + 512 * (jj + 1)],
                                 start=(k == 0), stop=(k == NK - 1))
        nc.scalar.activation(dst[:, 1024 * jp:1024 * (jp + 1)], ps[:],
                             IDENT, bias=b_sb[:])

    for jp in range(S // 1024):
        qk_proj_block(wk_sb, bk_sb, kT_sb, jp)

    # v projection in normal orientation [s, c], split per head into the
    # DoubleRow pair layout (fp8) with trailing ones columns.
    for h in range(HL):
        nc.vector.memset(vext[h][:], 1.0)
    for tp in range(NPAIR):
        ps = pmm.tile([128, 1024], F32, tag="mm")
        for tt in range(2):
            t = 2 * tp + tt
            for k in range(NK):
                nc.tensor.matmul(ps[:, 512 * tt:512 * tt + CW],
                                 xT_sb[k][:, 128 * t:128 * (t + 1)], wv_sb[k][:],
                                 start=(k == 0), stop=(k == NK - 1))
        for tt in range(2):
            for h in range(HL):
                nc.vector.tensor_copy(vext[h][:, tp, tt, 0:Dh],
                                      ps[:, 512 * tt + Dh * h:512 * tt + Dh * (h + 1)])

    for jp in range(S // 1024):
        qk_proj_block(wq_sb, bq_sb, qT_sb, jp)

    # Phase C: streaming attention + interleaved output projection.
    # Eviction copies alternate DVE/ACT; output DMAs are partition-split
    # across two queues.
    def out_proj_prev(jb_):
        for st in range(4):
            sq0 = 512 * jb_ + 128 * st
            pf = pmm.tile([128, 1024], F32, tag="mm", name="pf")
            nc.tensor.matmul(pf[:, 0:512], onormT[:, sq0:sq0 + 128], wo_sb[:],
                             start=True, stop=True)
            ob = outp.tile([128, 512], F32, tag="ob")
            if (jb_ * 4 + st) % 2 == 0:
                nc.vector.tensor_copy(ob[:], pf[:, 0:512])
            else:
                nc.scalar.copy(ob[:], pf[:, 0:512])
            nc.sync.dma_start(out[sq0:sq0 + 64, :], ob[0:64, :])
            nc.sync.dma_start(out[sq0 + 64:sq0 + 128, :], ob[64:128, :])

    tile_idx = 0
    for jb in range(NSQ):
        q0 = 512 * jb
        po = {h: pacc.tile([VW, 512], F32, tag=f"acc{h}", name=f"po{h}")
              for h in range(HL)}

        def emit_pv(e_tiles, p):
            for h in range(HL):
                nc.tensor.matmul(
                    po[h][:],
                    vext[h][:, p, :, 0:VW],
                    e_tiles[h][:].rearrange("p (a b) -> p a b", a=2),
                    start=(p == 0), stop=(p == NPAIR - 1),
                    perf_mode=DR)

        e_prev = None
        for p in range(NPAIR):
            s = {h: pmm.tile([128, 1024], F32, tag="mm", name=f"s{h}")
                 for h in range(HL)}
            # score MMs: heads alternate so their row-disjoint (64-row)
            # MMs co-execute in the PE array.
            for i in range(2):
                c = 2 * p + i
                for h in range(HL):
                    nc.tensor.matmul(s[h][:, 512 * i:512 * (i + 1)],
                                     kT_sb[Dh * h:Dh * (h + 1), 128 * c:128 * (c + 1)],
                                     qT_sb[Dh * h:Dh * (h + 1), q0:q0 + 512],
                                     start=True, stop=True)
            e_cur = {}
            for h in range(HL):
                e_cur[h] = expp.tile([128, 1024], FP8, tag=f"e{h}", bufs=3,
                                     name=f"e{h}")
                eng = PATTERN[tile_idx % len(PATTERN)]
                tile_idx += 1
                if eng == "A":
                    nc.scalar.activation(e_cur[h][:], s[h][:], EXP, scale=float(SCALE))
                else:
                    nc.vector.tensor_scalar(e_cur[h][:].bitcast(I8), s[h][:],
                                            A8, B8, ALU.mult, ALU.add)
            # software-pipelined: PV for pair p-1 lands after the scores+exp
            # of pair p so exp latency hides behind the next pair's MMs.
            if e_prev is not None:
                emit_pv(e_prev, p - 1)
            e_prev = e_cur
        emit_pv(e_prev, NPAIR - 1)

        if jb > 0:
            out_proj_prev(jb - 1)

        for h in range(HL):
            # NB: reciprocal_approx_* mis-executes at base partition != 0;
            # copy the denominator row (partition 64) to partition 0 first.
            r0 = rp.tile([1, 512], F32, tag="r0")
            nc.vector.tensor_copy(r0[:], po[h][Dh:VW, :])
            r = rp.tile([1, 512], F32, tag="r")
            nc.vector.reciprocal_approx_fast(r[:], r0[:])
            rb = rp.tile([Dh, 512], F32, tag="rb")
            nc.gpsimd.partition_broadcast(rb[:], r[:])
            nc.vector.tensor_mul(onormT[Dh * h:Dh * (h + 1), q0:q0 + 512],
                                 po[h][0:Dh, :], rb[:])
        if jb == NSQ - 1:
            out_proj_prev(jb)

    if dbg:
        for name, sb in (("qT", qT_sb), ("kT", kT_sb)):
            if name in dbg:
                nc.sync.dma_start(dbg[name][:, :], sb[:])


def build_nc():
    nc = bacc.Bacc("TRN2", target_bir_lowering=False, debug=False,
                   enable_asserts=False, num_devices=N_CORES)
    NK_ = D // 128
    io = {
        "xT": nc.dram_tensor("xT", [D, S], BF16, kind="ExternalInput").ap(),
        "wq": nc.dram_tensor("wq", [128, NK_ * CW], BF16, kind="ExternalInput").ap(),
        "wk": nc.dram_tensor("wk", [128, NK_ * CW], BF16, kind="ExternalInput").ap(),
        "wv": nc.dram_tensor("wv", [128, NK_ * CW], BF16, kind="ExternalInput").ap(),
        "wo": nc.dram_tensor("wo", [CW, D], BF16, kind="ExternalInput").ap(),
        "bq": nc.dram_tensor("bq", [CW, 1], F32, kind="ExternalInput").ap(),
        "bk": nc.dram_tensor("bk", [CW, 1], F32, kind="ExternalInput").ap(),
        "out": nc.dram_tensor("out", [S, D], F32, kind="ExternalOutput").ap(),
    }
    with tile.TileContext(nc) as tc, ExitStack() as ctx:
        _build_body(ctx, tc, io)
    nc.compile()
    return nc


def _shuf_w(w):
    # [D, CW] -> [128, NK*CW]: row 128k+p lands at [p, k*CW:(k+1)*CW]
    nk = w.shape[0] // 128
    return np.ascontiguousarray(
        w.reshape(nk, 128, -1).transpose(1, 0, 2).reshape(128, -1))


def make_in_maps(x, wq, bq, wk, bk, wv, bv, wo, bo):
    """Shard the full inputs across the 8 cores (host-side marshalling)."""
    bf16 = ml_dtypes.bfloat16
    in_maps = []
    for c in range(N_CORES):
        b, hp = divmod(c, 4)
        cs = slice(CW * hp, CW * (hp + 1))
        xT = np.ascontiguousarray(x[b, :S].T).astype(bf16)
        in_maps.append({
            "xT": xT,
            "wq": _shuf_w(wq[:, cs]).astype(bf16),
            "wk": _shuf_w(wk[:, cs]).astype(bf16),
            "wv": _shuf_w(wv[:, cs]).astype(bf16),
            "wo": np.ascontiguousarray(wo[cs, :]).astype(bf16),
            "bq": np.ascontiguousarray(bq[cs].reshape(CW, 1)).astype(np.float32),
            "bk": np.ascontiguousarray(bk[cs].reshape(CW, 1)).astype(np.float32),
        })
    return in_maps


_CACHE = {}


def _get_nc():
    if "nc" not in _CACHE:
        _CACHE["nc"] = build_nc()
    return _CACHE["nc"]


def run_sharded(nc, in_maps, **kwargs):
    return run_bass_kernel_spmd(nc, in_maps, core_ids=list(range(N_CORES)), **kwargs)


def gather(results, bv, wo, bo):
    # v is projected without bias on-device; softmax rows sum to 1, so
    # out = device_out_sum + (bv @ wo + bo).
    bias_row = (bv.astype(np.float64) @ wo.astype(np.float64)
                + bo.astype(np.float64)).astype(np.float32)
    out = np.zeros((B, S, D), np.float32)
    for c in range(N_CORES):
        out[c // 4] += results[c]["out"]
    out += bias_row
    return out


def kernel(x, wq, bq, wk, bk, wv, bv, wo, bo):
    x, wq, bq, wk, bk, wv, bv, wo, bo = (
        np.asarray(a, np.float32) for a in (x, wq, bq, wk, bk, wv, bv, wo, bo))
    nc = _get_nc()
    in_maps = make_in_maps(x, wq, bq, wk, bk, wv, bv, wo, bo)
    res = run_sharded(nc, in_maps)
    return gather(res.results, bv, wo, bo)
